# revision 17
# baseline (speedup 1.0000x reference)
"""CRF depth upsampler v2 — Bass/Tile kernel for 8 Trainium2 NeuronCores.

Sharding: 8 shards = 4 images x 2 width-halves (owned 320 cols + halo, padded
to WS=384); rows padded 480 -> 504 = 8 groups of 63 (G2), 72 L18 tiles, 4 D1
blocks of 126.

Math: label-compat matrix mu commuted through the linear guided filter and
factorized at rank K=2 (end-to-end err ~5.5e-3 vs reference, tol 2e-2); the
guided filter runs on 2 channels packed as partition p = h*2+k over 63-row
groups.

Host precomputes: bilinear up, unary cu (fp16) and E0=exp(cu) (bf16) in
G-layout [NG,126,9*WS], plus G-replicated img/invn planes, so every per-group
DMA is a contiguous 2D transfer.  Device: setup computes guided-filter image
stats (mi, Sigma^-1/N) via box filters (H via banded TensorE matmuls, W via
VectorE cumsum+diff) and stores them fp16 in G-layout; two mean-field
iterations run as a 3-stage software pipeline (A: softmax contraction+scans,
B: covariance/a/b, C: second box + message + exp), with the final expectation
fused into iteration 2's C stage.
"""
import sys
import numpy as np
import ml_dtypes
from contextlib import ExitStack

sys.path.insert(0, "/opt/trn_rl_repo")
import concourse.bass as bass
import concourse.bacc as bacc
import concourse.tile as tile
from concourse import mybir
from concourse.bass_utils import run_bass_kernel_spmd

F32 = mybir.dt.float32
F32R = mybir.dt.float32r
F16 = mybir.dt.float16
BF16 = mybir.dt.bfloat16
AF = mybir.ActivationFunctionType
ALU = mybir.AluOpType
BF16NP = ml_dtypes.bfloat16

RAD = 15
NITERS = 2
EPS = np.float32(0.01)
GAMMA = np.float32(0.05)
NL = 18
K = 2
B, C, H, W = 4, 3, 480, 640
HP = 504
WS = 384
GR = 63          # rows per G2 group
NG = 8
NTG = 9          # L18 tiles per group
ND = 4           # D1 blocks of 126
PAD = RAD + 1
SXW = PAD + WS + RAD
SHARD_OFF = [0, 256]
OWN = [(0, 0, 320), (320, 64, 320)]
IIPAIRS = [(0, 0), (0, 1), (0, 2), (1, 1), (1, 2), (2, 2)]
PSETS = ["I0", "I1", "I2"] + [f"II{a}{b}" for (a, b) in IIPAIRS]


# ---------------- host math ----------------
def _interp_mat(n_in, n_out):
    scale = n_in / n_out
    coords = (np.arange(n_out, dtype=np.float64) + 0.5) * scale - 0.5
    lo = np.floor(coords).astype(int)
    frac = coords - lo
    m = np.zeros((n_out, n_in), dtype=np.float64)
    for i in range(n_out):
        l0 = min(max(lo[i], 0), n_in - 1)
        l1 = min(max(lo[i] + 1, 0), n_in - 1)
        m[i, l0] += 1 - frac[i]
        m[i, l1] += frac[i]
    return m.astype(np.float32)


def bilinear_up(x, out_h, out_w):
    mh = _interp_mat(x.shape[-2], out_h)
    mw = _interp_mat(x.shape[-1], out_w)
    out = np.einsum('oh,...hw->...ow', mh, x.astype(np.float32))
    out = np.einsum('ow,...hw->...ho', mw, out)
    return out.astype(np.float32)


def build_constants(maxd):
    labels = np.linspace(np.float32(0.0), np.float32(maxd), NL).astype(np.float32)
    mu = np.sqrt((labels[:, None] - labels[None, :]) ** 2 + GAMMA ** 2).astype(np.float32)
    U, S, Vt = np.linalg.svd(mu.astype(np.float64))
    Vk = (Vt[:K].T).astype(np.float32)
    Usig = (U[:, :K] * S[:K]).astype(np.float32)
    return labels, mu, Vk, Usig


def make_invN_shard(off):
    ys = np.arange(HP)
    xs = np.arange(off, off + WS)
    cy = np.minimum(ys + RAD, H - 1) - np.maximum(ys - RAD, 0) + 1
    cy[ys >= H] = 1
    cx = np.minimum(xs + RAD, W - 1) - np.maximum(xs - RAD, 0) + 1
    cx = np.maximum(cx, 1)
    n = cy[:, None].astype(np.float32) * cx[None, :].astype(np.float32)
    return (np.float32(1.0) / n).astype(np.float32)


# ---------------- stationary matrices ----------------
def _z():
    return np.zeros((126, 128), np.float32)


def build_stationaries(labels, Vk, Usig):
    S = {}
    for j in range(NTG):        # L18 tile j in group -> G2 y_k = sum_l Vk[l,k] E_l
        m = _z()
        for h in range(7):
            for l in range(NL):
                for k in range(K):
                    m[h * NL + l, (j * 7 + h) * K + k] = Vk[l, k]
        S[("VC", j)] = m
    for j in range(NTG):        # -> G2 zr (sum_l, replicated over k)
        m = _z()
        for h in range(7):
            for l in range(NL):
                for k in range(K):
                    m[h * NL + l, (j * 7 + h) * K + k] = 1.0
        S[("ZR", j)] = m
    for j in range(NTG):        # G2 -> L18 tile j: msg_l = sum_k Usig[l,k] qf_k
        m = _z()
        for h in range(7):
            for l in range(NL):
                for k in range(K):
                    m[(j * 7 + h) * K + k, h * NL + l] = Usig[l, k]
        S[("UX", j)] = m
    for j in range(NTG):        # L18 tile j -> packed num(p=r)/den(p=63+r)
        m = _z()
        for h in range(7):
            r = j * 7 + h
            for l in range(NL):
                m[h * NL + l, r] += labels[l]
                m[h * NL + l, 64 + r] += 1.0
        S[("FING", j)] = m
    for dg in (-1, 0, 1):       # G2 banded box-H (dg = g_in - g_out)
        m = _z()
        for hi in range(GR):
            for ho in range(GR):
                if abs(dg * GR + hi - ho) <= RAD:
                    for k in range(K):
                        m[hi * K + k, ho * K + k] = 1.0
        S[("BOXH", dg)] = m
    for dg in (0, 1):           # variant when g_in == NG-1 (rows >= 480 invalid)
        m = _z()
        for hi in range(GR):
            if GR * (NG - 1) + hi >= H:
                continue
            for ho in range(GR):
                if abs(dg * GR + hi - ho) <= RAD:
                    for k in range(K):
                        m[hi * K + k, ho * K + k] = 1.0
        S[("BOXHE", dg)] = m
    for dg in (-1, 0, 1):
        S[("BOXHN", dg)] = -S[("BOXH", dg)]
    for dg in (0, 1):
        S[("BOXHNE", dg)] = -S[("BOXHE", dg)]
    S[("ID",)] = np.concatenate([np.eye(126, dtype=np.float32), np.zeros((126, 2), np.float32)], axis=1)
    for dd in (-1, 0, 1):       # D1 banded box-H for setup planes (zero-padded data)
        m = _z()
        for hi in range(126):
            for ho in range(126):
                if abs(dd * 126 + hi - ho) <= RAD:
                    m[hi, ho] = 1.0
        S[("BOXHD", dd)] = m
    return S


STATION_ORDER = ([("VC", j) for j in range(NTG)] + [("ZR", j) for j in range(NTG)]
                 + [("UX", j) for j in range(NTG)] + [("FING", j) for j in range(NTG)]
                 + [("BOXH", d) for d in (-1, 0, 1)] + [("BOXHE", d) for d in (0, 1)]
                 + [("BOXHN", d) for d in (-1, 0, 1)] + [("BOXHNE", d) for d in (0, 1)]
                 + [("ID",)] + [("BOXHD", d) for d in (-1, 0, 1)])
NSTAT = len(STATION_ORDER)
ST_BF16 = {("VC",), ("ZR",)}    # by key[0] prefix; FING/UX/BOX*/ID stay f32r


def st_dtype(key):
    return BF16 if key[0] in ("VC", "ZR") else F32R


# ---------------- device program ----------------
def build_program(debug=False):
    nc = bacc.Bacc("TRN2", target_bir_lowering=False, debug=False)

    img_in = nc.dram_tensor("img", [C, HP, WS], F32, kind="ExternalInput").ap()
    invn_in = nc.dram_tensor("invn", [HP, WS], F32, kind="ExternalInput").ap()
    imgg_in = nc.dram_tensor("imgg", [NG, 126, C * WS], F32, kind="ExternalInput").ap()
    invng_in = nc.dram_tensor("invng", [NG, 126, WS], F32, kind="ExternalInput").ap()
    cug_in = nc.dram_tensor("cug", [NG, 126, NTG * WS], F16, kind="ExternalInput").ap()
    e0g_in = nc.dram_tensor("e0g", [NG, 126, NTG * WS], BF16, kind="ExternalInput").ap()
    nst = nc.dram_tensor("stations", [NSTAT, 126, 128], F32, kind="ExternalInput").ap()
    out_d = nc.dram_tensor("out", [H, WS], F32, kind="ExternalOutput").ap()

    mig_d = nc.dram_tensor("mig_d", [NG, 126, C * WS], F16).ap()
    sig_d = nc.dram_tensor("sig_d", [NG, 126, 9 * WS], F16).ap()
    e2g_d = nc.dram_tensor("e2g_d", [NG, 126, NTG * WS], BF16,
                           kind="ExternalOutput" if debug else "Internal").ap()

    dbg = {}
    if debug:
        for name, fw in [("dbg_z", WS), ("dbg_mz", WS), ("dbg_cov", C * WS),
                         ("dbg_a", C * WS), ("dbg_bt", WS), ("dbg_qf", WS),
                         ("dbg_q", NTG * WS), ("dbg_z2", WS), ("dbg_q2", NTG * WS)]:
            dbg[name] = nc.dram_tensor(name, [NG, 126, fw], F32, kind="ExternalOutput").ap()
        dbg["dbg_e2"] = nc.dram_tensor("dbg_e2", [NG, 126, NTG * WS], BF16, kind="ExternalOutput").ap()

    def dram_ap(t, off, dims):
        return bass.AP(t.tensor, t.offset + off, dims)

    def bcast(ap_, n, pos=1):
        """insert a step-0 dim of size n into an AP's free dims at pos"""
        dims = [list(d) for d in ap_.ap]
        dims.insert(pos, [0, n])
        return bass.AP(ap_.tensor, ap_.offset, dims)

    with tile.TileContext(nc) as tc, ExitStack() as ctx:
        con = ctx.enter_context(tc.tile_pool(name="con", bufs=1))
        stg = ctx.enter_context(tc.tile_pool(name="stg", bufs=2))
        psT = ctx.enter_context(tc.tile_pool(name="psT", bufs=6, space="PSUM"))
        psA = ctx.enter_context(tc.tile_pool(name="psA", bufs=2, space="PSUM"))

        st_tiles = {}

        def st(key):
            if key not in st_tiles:
                idx = STATION_ORDER.index(key)
                r = stg.tile([126, 128], F32, tag="str")
                nc.sync.dma_start(r[:], nst[idx])
                t = con.tile([126, 128], st_dtype(key), tag=f"st{idx}")
                nc.vector.tensor_copy(t[:], r[:])
                st_tiles[key] = t
            return st_tiles[key]

        def mm(psum_ap, key, rhs_ap, start, stop, cols=126):
            nc.tensor.matmul(psum_ap, st(key)[:, 0:cols], rhs_ap, start=start, stop=stop)

        dumb = con.tile([126, WS], F32, tag="dumb")
        nc.vector.memset(dumb[:], 0.0)

        # =================== SETUP ===================
        with ExitStack() as sctx:
            sp2 = sctx.enter_context(tc.tile_pool(name="sp2", bufs=1))
            sp3 = sctx.enter_context(tc.tile_pool(name="sp3", bufs=3))
            sc1 = sctx.enter_context(tc.tile_pool(name="sc1", bufs=1))

            imgA = sc1.tile([126, ND * C * WS], F32, tag="imgA")
            for c in range(C):
                dst = bass.AP(imgA[:].tensor, imgA[:].offset + c * WS,
                              [list(imgA[:].ap[0]), [C * WS, ND], [1, WS]])
                nc.sync.dma_start(dst, dram_ap(img_in, c * HP * WS,
                                               [[WS, 126], [126 * WS, ND], [1, WS]]))
            imgAr = sc1.tile([126, ND * C * WS], F32R, tag="imgAr")
            nc.vector.tensor_copy(imgAr[:], imgA[:])
            invnA = sc1.tile([126, ND * WS], F32, tag="invnA")
            nc.sync.dma_start(invnA[:], dram_ap(invn_in, 0,
                [[WS, 126], [126 * WS, ND], [1, WS]]))

            def img_sl(d, c):
                return imgA[:, (d * C + c) * WS:(d * C + c + 1) * WS]

            prods = {p: [None] * ND for p in IIPAIRS}
            for dl in range(ND + 1):
                if dl < ND:
                    for (a, b_) in IIPAIRS:
                        t = sp3.tile([126, WS], F32R, tag=f"prod{a}{b_}")
                        nc.vector.tensor_tensor(t[:], img_sl(dl, a), img_sl(dl, b_), ALU.mult)
                        prods[(a, b_)][dl] = t
                if dl < 1:
                    continue
                d = dl - 1
                dds = [dd for dd in (-1, 0, 1) if 0 <= d + dd < ND]
                boxP = {}
                for pset in PSETS:
                    hb = psT.tile([126, WS], F32, tag="ps")
                    for i, dd in enumerate(dds):
                        if pset.startswith("II"):
                            rhs = prods[(int(pset[2]), int(pset[3]))][d + dd][:]
                        else:
                            cc = int(pset[1])
                            rhs = imgAr[:, ((d + dd) * C + cc) * WS:((d + dd) * C + cc + 1) * WS]
                        mm(hb[:], ("BOXHD", dd), rhs, i == 0, i == len(dds) - 1)
                    sat = sp2.tile([126, WS], F32, tag="sat")
                    nc.vector.tensor_tensor_scan(sat[:], hb[:], dumb[:], 0.0, ALU.add, ALU.bypass)
                    bx = sp2.tile([126, WS], F32, tag=f"bx{pset}")
                    # box-W diff with edge clamping
                    nc.vector.tensor_copy(bx[:, 0:RAD + 1], sat[:, RAD:2 * RAD + 1])
                    nc.vector.tensor_tensor(bx[:, RAD + 1:WS - RAD], sat[:, 2 * RAD + 1:WS],
                                            sat[:, 0:WS - 2 * RAD - 1], ALU.subtract)
                    last = sat[:, WS - 1:WS]
                    lastb = bass.AP(last.tensor, last.offset, [list(last.ap[0]), [0, RAD]])
                    nc.vector.tensor_tensor(bx[:, WS - RAD:WS], lastb,
                                            sat[:, WS - 2 * RAD - 1:WS - RAD - 1], ALU.subtract)
                    boxP[pset] = bx

                invd = invnA[:, d * WS:(d + 1) * WS]
                mi = sp2.tile([126, C * WS], F32, tag="mi")
                mi16 = sp2.tile([126, C * WS], F16, tag="mi16")
                for c in range(C):
                    nc.vector.tensor_tensor(mi[:, c * WS:(c + 1) * WS], boxP[f"I{c}"][:], invd, ALU.mult)
                nc.vector.tensor_copy(mi16[:], mi[:])

                sg = {}
                for (a, b_) in IIPAIRS:
                    u = sp2.tile([126, WS], F32, tag="sg_u")
                    nc.vector.tensor_tensor(u[:], boxP[f"II{a}{b_}"][:], invd, ALU.mult)
                    t1 = sp2.tile([126, WS], F32, tag="sg_t1")
                    nc.vector.tensor_tensor(t1[:], mi[:, a * WS:(a + 1) * WS],
                                            mi[:, b_ * WS:(b_ + 1) * WS], ALU.mult)
                    s = sp2.tile([126, WS], F32, tag=f"sg{a}{b_}")
                    nc.vector.tensor_tensor(s[:], u[:], t1[:], ALU.subtract)
                    if a == b_:
                        nc.vector.tensor_scalar(s[:], s[:], float(EPS), None, ALU.add)
                    sg[(a, b_)] = s

                def sget(a, b_):
                    return sg[(a, b_)] if (a, b_) in sg else sg[(b_, a)]

                cof = {}
                for (i, j, a1, b1, a2, b2) in [
                        (0, 0, (1, 1), (2, 2), (1, 2), (1, 2)),
                        (0, 1, (0, 2), (1, 2), (0, 1), (2, 2)),
                        (0, 2, (0, 1), (1, 2), (0, 2), (1, 1)),
                        (1, 1, (0, 0), (2, 2), (0, 2), (0, 2)),
                        (1, 2, (0, 1), (0, 2), (0, 0), (1, 2)),
                        (2, 2, (0, 0), (1, 1), (0, 1), (0, 1))]:
                    t1 = sp2.tile([126, WS], F32, tag="cf_t1")
                    nc.vector.tensor_tensor(t1[:], sget(*a1)[:], sget(*b1)[:], ALU.mult)
                    t2 = sp2.tile([126, WS], F32, tag="cf_t2")
                    nc.vector.tensor_tensor(t2[:], sget(*a2)[:], sget(*b2)[:], ALU.mult)
                    cf = sp2.tile([126, WS], F32, tag=f"cf{i}{j}")
                    nc.vector.tensor_tensor(cf[:], t1[:], t2[:], ALU.subtract)
                    cof[(i, j)] = cf

                det = sp2.tile([126, WS], F32, tag="det")
                nc.vector.tensor_tensor(det[:], sget(0, 0)[:], cof[(0, 0)][:], ALU.mult)
                for (i, j) in [(0, 1), (0, 2)]:
                    t1 = sp2.tile([126, WS], F32, tag="det_t")
                    nc.vector.tensor_tensor(t1[:], sget(i, j)[:], cof[(i, j)][:], ALU.mult)
                    nc.vector.tensor_tensor(det[:], det[:], t1[:], ALU.add)
                idet = sp2.tile([126, WS], F32, tag="idet")
                nc.vector.reciprocal_approx_fast(idet[:], det[:])
                nc.vector.tensor_tensor(idet[:], idet[:], invd, ALU.mult)
                sv16 = sp2.tile([126, 9 * WS], F16, tag="sv16")
                for ci in range(3):
                    for cj in range(3):
                        key = (ci, cj) if (ci, cj) in cof else (cj, ci)
                        nc.vector.tensor_tensor(
                            sv16[:, (ci * 3 + cj) * WS:(ci * 3 + cj + 1) * WS],
                            cof[key][:], idet[:], ALU.mult)

                # write G-form (rep over k): groups 2d, 2d+1
                for gp in range(2):
                    g = 2 * d + gp
                    sl = mi16[63 * gp:63 * gp + 63, :]
                    src = bass.AP(sl.tensor, sl.offset, [list(sl.ap[0]), [0, K]] + [list(x) for x in sl.ap[1:]])
                    nc.scalar.dma_start(dram_ap(mig_d, g * 126 * C * WS,
                        [[K * C * WS, GR], [C * WS, K], [1, C * WS]]), src)
                    sl2 = sv16[63 * gp:63 * gp + 63, :]
                    src2 = bass.AP(sl2.tensor, sl2.offset, [list(sl2.ap[0]), [0, K]] + [list(x) for x in sl2.ap[1:]])
                    nc.scalar.dma_start(dram_ap(sig_d, g * 126 * 9 * WS,
                        [[K * 9 * WS, GR], [9 * WS, K], [1, 9 * WS]]), src2)

        # =================== ITERATIONS ===================
        wE = ctx.enter_context(tc.tile_pool(name="wE", bufs=2))
        wcu = ctx.enter_context(tc.tile_pool(name="wcu", bufs=2))
        wIr = ctx.enter_context(tc.tile_pool(name="wIr", bufs=3))
        wrep = ctx.enter_context(tc.tile_pool(name="wrep", bufs=2))
        wsat = ctx.enter_context(tc.tile_pool(name="wsat", bufs=3))
        w1 = ctx.enter_context(tc.tile_pool(name="w1", bufs=1))
        w2 = ctx.enter_context(tc.tile_pool(name="w2", bufs=2))

        def scan_sat_seg(dst, seg, src_ap):
            """fill dst[:, seg*SXW : (seg+1)*SXW] with padded cumsum of src"""
            o = seg * SXW
            nc.vector.memset(dst[:, o:o + PAD].bitcast(F32), 0.0)
            nc.vector.tensor_tensor_scan(dst[:, o + PAD:o + PAD + WS], src_ap,
                                         dumb[:], 0.0, ALU.add, ALU.bypass)
            last = dst[:, o + PAD + WS - 1:o + PAD + WS]
            lastb = bass.AP(last.tensor, last.offset, [list(last.ap[0]), [0, RAD]])
            nc.vector.tensor_copy(dst[:, o + PAD + WS:o + SXW], lastb)

        def emit_boxh(out_ps, sats, g, seg=0, start=True, stop=True):
            pieces = []
            for dg in (-1, 0, 1):
                gi = g + dg
                if not 0 <= gi < NG:
                    continue
                pk = ("BOXHE", dg) if gi == NG - 1 else ("BOXH", dg)
                nk = ("BOXHNE", dg) if gi == NG - 1 else ("BOXHN", dg)
                s = sats[gi]
                o = seg * SXW
                pieces.append((pk, s[:, o + PAD + RAD:o + PAD + RAD + WS]))
                pieces.append((nk, s[:, o:o + WS]))
            for i, (key, rap) in enumerate(pieces):
                mm(out_ps, key, rap, start and i == 0, stop and i == len(pieces) - 1)

        for it in range(NITERS):
            Dz = [None] * NG
            Dp = [None] * NG
            Da = [None] * NG
            Db = [None] * NG
            Ir = [None] * NG
            Nr = [None] * NG
            CU = [None] * NG
            e_src = e0g_in if it == 0 else e2g_d

            for gl in range(NG + 2):
                # ---------- stage A ----------
                g = gl
                if g < NG:
                    Eg = wE.tile([126, NTG * WS], BF16, tag="E")
                    nc.sync.dma_start(Eg[:], dram_ap(e_src, g * 126 * NTG * WS,
                                                     [[NTG * WS, 126], [1, NTG * WS]]))
                    Ir[g] = wIr.tile([126, C * WS], F32, tag="Ir", name="Irt")
                    nc.sync.dma_start(Ir[g][:], dram_ap(imgg_in, g * 126 * C * WS,
                                                        [[C * WS, 126], [1, C * WS]]))
                    zp = psT.tile([126, WS], F32, tag="ps")
                    for j in range(NTG):
                        mm(zp[:], ("ZR", j), Eg[:, j * WS:(j + 1) * WS], j == 0, j == NTG - 1)
                    yp = psT.tile([126, WS], F32, tag="ps")
                    for j in range(NTG):
                        mm(yp[:], ("VC", j), Eg[:, j * WS:(j + 1) * WS], j == 0, j == NTG - 1)
                    izr = w2.tile([126, WS], F32, tag="izr")
                    nc.vector.reciprocal_approx_fast(izr[:], zp[:])
                    z = w2.tile([126, WS], F32, tag="z")
                    nc.vector.tensor_tensor(z[:], yp[:], izr[:], ALU.mult)
                    if debug:
                        nc.scalar.dma_start(dram_ap(dbg["dbg_z" if it == 0 else "dbg_z2"], g * 126 * WS,
                                                    [[WS, 126], [1, WS]]), z[:])
                    Dz[g] = wsat.tile([126, SXW], F32R, tag="Dz", name="Dzt")
                    scan_sat_seg(Dz[g], 0, z[:])
                    pc = w1.tile([126, C * WS], F32, tag="pc")
                    nc.vector.tensor_tensor(pc[:], bcast(z[:], C), Ir[g][:], ALU.mult)
                    Dp[g] = wsat.tile([126, C * SXW], F32R, tag="Dp", name="Dpt")
                    for c in range(C):
                        scan_sat_seg(Dp[g], c, pc[:, c * WS:(c + 1) * WS])

                # ---------- stage B ----------
                g = gl - 1
                if 0 <= g < NG:
                    CU[g] = wcu.tile([126, NTG * WS], F16, tag="cu", name="CUt")
                    nc.scalar.dma_start(CU[g][:], dram_ap(cug_in, g * 126 * NTG * WS,
                                                          [[NTG * WS, 126], [1, NTG * WS]]))
                    si = wrep.tile([126, 9 * WS], F16, tag="si")
                    nc.sync.dma_start(si[:], dram_ap(sig_d, g * 126 * 9 * WS,
                                                     [[9 * WS, 126], [1, 9 * WS]]))
                    mir = wrep.tile([126, C * WS], F16, tag="mir")
                    nc.scalar.dma_start(mir[:], dram_ap(mig_d, g * 126 * C * WS,
                                                        [[C * WS, 126], [1, C * WS]]))
                    Nr[g] = wrep.tile([126, WS], F32, tag="nr", name="Nrt")
                    nc.scalar.dma_start(Nr[g][:], dram_ap(invng_in, g * 126 * WS,
                                                          [[WS, 126], [1, WS]]))
                    mzp = psT.tile([126, WS], F32, tag="ps")
                    emit_boxh(mzp[:], Dz, g)
                    corr = []
                    for c in range(C):
                        cp = psT.tile([126, WS], F32, tag="ps")
                        emit_boxh(cp[:], Dp, g, seg=c)
                        corr.append(cp)
                    if debug and it == 0:
                        mzc = w2.tile([126, WS], F32, tag="mzc")
                        nc.scalar.copy(mzc[:], mzp[:])
                        nc.scalar.dma_start(dram_ap(dbg["dbg_mz"], g * 126 * WS,
                                                    [[WS, 126], [1, WS]]), mzc[:])
                    tc_ = w1.tile([126, C * WS], F32, tag="tc")
                    nc.vector.tensor_tensor(tc_[:], bcast(mzp[:], C), mir[:], ALU.mult)
                    cov = w1.tile([126, C * WS], F32, tag="cov")
                    for c in range(C):
                        nc.vector.tensor_tensor(cov[:, c * WS:(c + 1) * WS], corr[c][:],
                                                tc_[:, c * WS:(c + 1) * WS], ALU.subtract)
                    v = w2.tile([126, WS], F32, tag="v")
                    nc.vector.tensor_tensor(v[:], mzp[:], Nr[g][:], ALU.mult)
                    if debug and it == 0:
                        nc.scalar.dma_start(dram_ap(dbg["dbg_cov"], g * 126 * C * WS,
                                                    [[C * WS, 126], [1, C * WS]]), cov[:])
                    pr = w1.tile([126, 9 * WS], F32R, tag="pr")
                    cov_b = bass.AP(cov[:].tensor, cov[:].offset,
                                    [list(cov[:].ap[0]), [0, C], [WS, C], [1, WS]])
                    nc.vector.tensor_tensor(pr[:], cov_b, si[:], ALU.mult)
                    aps = []
                    for c in range(C):
                        at = psT.tile([126, WS], F32, tag="ps")
                        for cp2 in range(C):
                            mm(at[:], ("ID",), pr[:, (c * 3 + cp2) * WS:(c * 3 + cp2 + 1) * WS],
                               cp2 == 0, cp2 == 2)
                        aps.append(at)
                    Da[g] = wsat.tile([126, C * SXW], F32R, tag="Da", name="Dat")
                    uc = w1.tile([126, C * WS], F32R, tag="uc")
                    for c in range(C):
                        scan_sat_seg(Da[g], c, aps[c][:])
                        nc.vector.tensor_tensor(uc[:, c * WS:(c + 1) * WS], aps[c][:],
                                                mir[:, c * WS:(c + 1) * WS], ALU.mult)
                    if debug and it == 0:
                        acp = w1.tile([126, C * WS], F32, tag="acp")
                        for c in range(C):
                            nc.scalar.copy(acp[:, c * WS:(c + 1) * WS], aps[c][:])
                        nc.scalar.dma_start(dram_ap(dbg["dbg_a"], g * 126 * C * WS,
                                                    [[C * WS, 126], [1, C * WS]]), acp[:])
                    sp = psT.tile([126, WS], F32, tag="ps")
                    for c in range(C):
                        mm(sp[:], ("ID",), uc[:, c * WS:(c + 1) * WS], c == 0, c == 2)
                    bt = w2.tile([126, WS], F32, tag="bt")
                    nc.vector.scalar_tensor_tensor(bt[:], sp[:], -1.0, v[:], ALU.mult, ALU.add)
                    if debug and it == 0:
                        nc.scalar.dma_start(dram_ap(dbg["dbg_bt"], g * 126 * WS,
                                                    [[WS, 126], [1, WS]]), bt[:])
                    Db[g] = wsat.tile([126, SXW], F32R, tag="Db", name="Dbt")
                    scan_sat_seg(Db[g], 0, bt[:])

                # ---------- stage C ----------
                g = gl - 2
                if 0 <= g < NG:
                    qp128 = psA.tile([128, WS], F32, tag="acc")
                    qp = qp128[0:126, :]
                    emit_boxh(qp, Db, g, stop=False)
                    wc = w1.tile([126, C * WS], F32R, tag="wc")
                    for c in range(C):
                        map_ = psT.tile([126, WS], F32, tag="ps")
                        emit_boxh(map_[:], Da, g, seg=c)
                        nc.vector.tensor_tensor(wc[:, c * WS:(c + 1) * WS], map_[:],
                                                Ir[g][:, c * WS:(c + 1) * WS], ALU.mult)
                    for c in range(C):
                        mm(qp, ("ID",), wc[:, c * WS:(c + 1) * WS], False, c == 2)
                    qf = w2.tile([126, WS], F32R, tag="qf")
                    nc.vector.tensor_tensor(qf[:], qp, Nr[g][:], ALU.mult)
                    if debug and it == 0:
                        qfc = w2.tile([126, WS], F32, tag="qfc")
                        nc.vector.tensor_copy(qfc[:], qf[:].bitcast(F32))
                        nc.scalar.dma_start(dram_ap(dbg["dbg_qf"], g * 126 * WS,
                                                    [[WS, 126], [1, WS]]), qfc[:])
                    if it == 0:
                        e2s = w1.tile([126, NTG * WS], BF16, tag="e2s")
                    else:
                        finp = psA.tile([128, WS], F32, tag="acc")
                    for j in range(NTG):
                        msgp = psT.tile([126, WS], F32, tag="ps")
                        mm(msgp[:], ("UX", j), qf[:], True, True)
                        qj = w2.tile([126, WS], F32, tag="qj")
                        nc.vector.scalar_tensor_tensor(qj[:], msgp[:], -1.0,
                                                       CU[g][:, j * WS:(j + 1) * WS],
                                                       ALU.mult, ALU.add)
                        if debug:
                            nc.scalar.dma_start(dram_ap(dbg["dbg_q" if it == 0 else "dbg_q2"], g * 126 * NTG * WS + j * WS,
                                                        [[NTG * WS, 126], [1, WS]]), qj[:])
                        if it == 0:
                            nc.scalar.activation(e2s[:, j * WS:(j + 1) * WS], qj[:], AF.Exp)
                        else:
                            e3 = w2.tile([126, WS], F32R, tag="e3")
                            nc.scalar.activation(e3[:], qj[:], AF.Exp)
                            mm(finp[:], ("FING", j), e3[:], j == 0, j == NTG - 1, cols=128)
                    if it == 0:
                        nc.sync.dma_start(dram_ap(e2g_d, g * 126 * NTG * WS,
                                                  [[NTG * WS, 126], [1, NTG * WS]]), e2s[:])
                        if debug:
                            nc.scalar.dma_start(dram_ap(dbg["dbg_e2"], g * 126 * NTG * WS,
                                                        [[NTG * WS, 126], [1, NTG * WS]]), e2s[:])
                    else:
                        dens = w2.tile([63, WS], F32, tag="dens")
                        nc.scalar.copy(dens[:], finp[64:127, :])
                        iden = w2.tile([63, WS], F32, tag="iden")
                        nc.vector.reciprocal_approx_fast(iden[:], dens[:])
                        ot = w2.tile([63, WS], F32, tag="ot")
                        nc.vector.tensor_tensor(ot[:], finp[0:63, :], iden[:], ALU.mult)
                        rows = min(GR, H - GR * g)
                        nc.scalar.dma_start(out_d[GR * g:GR * g + rows, :], ot[:rows, :])

    nc.compile()
    return nc


# ---------------- host driver ----------------
_CACHE = {}


def _get_program(debug=False):
    key = ("prog", debug)
    if key not in _CACHE:
        _CACHE[key] = build_program(debug)
    return _CACHE[key]


def _g_layout(x):
    """[HP, NL, WS] -> [NG, 126, NTG*WS] with p=h*NL+l, f=j*WS+x"""
    # rows r = 63g + 7j + h
    x = x.reshape(NG, NTG, 7, NL, WS)            # g, j, h, l, x
    x = x.transpose(0, 2, 3, 1, 4)               # g, h, l, j, x
    return np.ascontiguousarray(x.reshape(NG, 126, NTG * WS))


def _g_rep(x, fw):
    """[HP, fw] -> [NG, 126, fw] with p = h*K+k (k replicated)"""
    x = x.reshape(NG, GR, fw)
    x = np.repeat(x, K, axis=1)
    return np.ascontiguousarray(x)


def make_core_inputs(inputs):
    disp = np.asarray(inputs['disp_lowres'], dtype=np.float32)
    img = np.asarray(inputs['img_highres'], dtype=np.float32)
    up_full = bilinear_up(disp[:, 0], H, W)
    maxd = float(up_full.max())
    labels, mu, Vk, Usig = build_constants(maxd)
    stats = build_stationaries(labels, Vk, Usig)
    st_arr = np.zeros((NSTAT, 126, 128), np.float32)
    for i, key in enumerate(STATION_ORDER):
        st_arr[i] = stats[key]
    in_maps = []
    for core in range(8):
        b, half = core // 2, core % 2
        off = SHARD_OFF[half]
        I = np.zeros((C, HP, WS), np.float32)
        I[:, :H] = img[b, :, :, off:off + WS]
        up = np.zeros((HP, WS), np.float32)
        up[:H] = up_full[b, :, off:off + WS]
        en = np.sqrt((up[:, None, :] - labels[None, :, None]) ** 2 + GAMMA ** 2)
        conf = (up > 0.01).astype(np.float32)
        cu = (conf[:, None, :] * np.float32(-10.0) * en).astype(np.float32)
        cu16 = cu.astype(np.float16)
        e0 = np.exp(cu16.astype(np.float32)).astype(BF16NP)
        invn = make_invN_shard(off)
        # G-layout / G-rep host prep
        imgg = np.concatenate([_g_rep(I[c], WS) for c in range(C)], axis=2)
        in_maps.append({
            "img": I,
            "invn": invn,
            "imgg": imgg,
            "invng": _g_rep(invn, WS),
            "cug": _g_layout(cu16),
            "e0g": _g_layout(e0),
            "stations": st_arr,
        })
    return in_maps


def kernel(**inputs):
    nc = _get_program(debug=False)
    in_maps = make_core_inputs(inputs)
    res = run_bass_kernel_spmd(nc, in_maps, list(range(8)))
    out = np.zeros((B, 1, H, W), np.float32)
    for core in range(8):
        b, half = core // 2, core % 2
        plane = res.results[core]["out"]
        g0, l0, n = OWN[half]
        out[b, 0, :, g0:g0 + n] = plane[:H, l0:l0 + n]
    return out
